# revision 1
# baseline (speedup 1.0000x reference)
"""GCN link predictor on 8 TRN2 NeuronCores.

Strategy (1D node partition, dst-sharded SPMM, pull-mode gathers):
  - x is replicated (transposed, bf16); every core computes the full
    h1 = x @ W1 gather table [100096, 128] bf16 (256B rows).
  - adjacency edges are sharded by dst owner (12500 nodes/core); within a
    core, edges are grouped by (128-row dst block, 32768-row src chunk) so
    each dma_gather call uses int16 indices against a single chunk base.
  - segment-sum = PE matmul with a per-tile one-hot built on DVE:
    onehot[e, d] = (iota[d] == dst_local[e]) * val[e]  (one fused
    tensor_scalar op per 128-edge tile), accumulated in PSUM per dst block.
  - layer 2: h2 = z1 @ W2 locally, AllGather h2 (bf16 [12500, 32]/core),
    expand into a 256B-row table, same SPMM.
  - z2 AllGathered (f32), expanded to a [100096, 64] f32 table; decoder
    edges are sharded by edge, grouped by (src chunk, dst chunk), two
    gathers + DVE mult/reduce per tile.
"""
import sys
import os

sys.path.insert(0, "/opt/trn_rl_repo")

import numpy as np
import ml_dtypes
from contextlib import ExitStack

from concourse import bass, bacc, tile, bass_utils
import concourse.mybir as mybir


def _install_ntff_hook():
    """Provide antenv.axon_hooks (missing in this image) so that
    run_bass_kernel_spmd(trace=True) can capture NTFF profiles via the
    axon PJRT .so — mirrors trn_agent_boot's ctypes shim."""
    if "antenv.axon_hooks" in sys.modules:
        return
    import types, ctypes, contextlib
    import antenv

    mod = types.ModuleType("antenv.axon_hooks")
    holder = {}
    mod.set_axon_ntff_profile_hook = lambda h: holder.__setitem__("h", h)
    mod.get_axon_ntff_profile_hook = lambda: holder.get("h")
    sys.modules["antenv.axon_hooks"] = mod
    antenv.axon_hooks = mod

    so_path = "/opt/axon/libaxon_pjrt.so"
    if not os.path.exists(so_path):
        return
    lib = ctypes.CDLL(so_path)
    if not hasattr(lib, "axon_start_nrt_profile"):
        return
    lib.axon_start_nrt_profile.argtypes = [ctypes.POINTER(ctypes.c_int64),
                                           ctypes.c_size_t]
    lib.axon_start_nrt_profile.restype = ctypes.c_int64
    lib.axon_stop_nrt_profile.argtypes = [ctypes.c_char_p]
    lib.axon_stop_nrt_profile.restype = ctypes.c_int64

    @contextlib.contextmanager
    def _hook(output_dir, device_ids):
        import jax
        jax.devices()
        if device_ids:
            ids = (ctypes.c_int64 * len(device_ids))(*device_ids)
            rc = lib.axon_start_nrt_profile(ids, len(device_ids))
        else:
            rc = lib.axon_start_nrt_profile(None, 0)
        if rc != 0:
            raise RuntimeError(f"axon_start_nrt_profile rc={rc}")
        try:
            yield
        finally:
            n = lib.axon_stop_nrt_profile(str(output_dir).encode())
            print(f"profile: {n} file(s) written to {output_dir}",
                  file=sys.stderr)

    mod.set_axon_ntff_profile_hook(_hook)


_install_ntff_hook()

F32 = mybir.dt.float32
BF16 = mybir.dt.bfloat16
I16 = mybir.dt.int16
BF = ml_dtypes.bfloat16

N_NODES = 100000
D_IN = 256
D_HID = 64
D_EMB = 32
ADJ_NNZ = 3200000
N_EDGES = 2000000
NCORE = 8
P_NODES = N_NODES // NCORE          # 12500
NODES_PAD = 782 * 128               # 100096
ZROWS = 98 * 128                    # 12544 (per-core z rows, padded)
NBLK = 98                           # dst blocks per core (last has 84 rows)
CHUNK = 32768
NCHUNK = 4
SUPER = 4                           # dst blocks per superblock
NSUPER = (NBLK + SUPER - 1) // SUPER
DEC_CALL_TILES = 32                 # decoder edges per gather call = 32*128

LAST_RESULT = None                  # BassKernelResults of the last run


def _wrap16(idx):
    """idx j -> partition j%16, col j//16, replicated to 128 partitions."""
    n = len(idx)
    assert n % 16 == 0
    a = idx.reshape(n // 16, 16).T
    return np.tile(a, (8, 1)).astype(np.int16)


def _prep_spmm(adj_src, adj_dst, adj_val):
    """Shard + sort adjacency; build per-core gather/meta streams.

    Returns (T_bc [98,4] common tile counts, per-core dict with idx_w,
    dst_meta, val_meta).
    """
    owner = adj_dst // P_NODES
    cores = []
    for m in range(NCORE):
        sel = owner == m
        src = adj_src[sel].astype(np.int64)
        ldst = (adj_dst[sel] - m * P_NODES).astype(np.int64)
        val = adj_val[sel]
        blk = ldst >> 7
        chk = src // CHUNK
        order = np.lexsort((chk, blk))
        src, ldst, val, blk, chk = (a[order] for a in (src, ldst, val, blk, chk))
        key = blk * NCHUNK + chk
        cnt = np.bincount(key, minlength=NBLK * NCHUNK).reshape(NBLK, NCHUNK)
        # segment start offsets in the sorted arrays
        starts = np.zeros(NBLK * NCHUNK + 1, np.int64)
        np.cumsum(cnt.ravel(), out=starts[1:])
        cores.append(dict(src=src, ldst=ldst, val=val, cnt=cnt, starts=starts))

    cnt_max = np.maximum.reduce([c["cnt"] for c in cores])
    T_bc = -(-cnt_max // 128)  # ceil
    NT = int(T_bc.sum())

    for c in cores:
        idx_stream = np.zeros(NT * 128, np.int16)
        dst_stream = np.zeros(NT * 128, np.float32)
        val_stream = np.zeros(NT * 128, np.float32)
        pos = 0
        for sb in range(NSUPER):
            blocks = range(sb * SUPER, min((sb + 1) * SUPER, NBLK))
            for ch in range(NCHUNK):
                for b in blocks:
                    t = int(T_bc[b, ch])
                    if t == 0:
                        continue
                    s = c["starts"][b * NCHUNK + ch]
                    e = c["starts"][b * NCHUNK + ch + 1]
                    n = e - s
                    idx_stream[pos:pos + n] = (c["src"][s:e] - ch * CHUNK).astype(np.int16)
                    dst_stream[pos:pos + n] = (c["ldst"][s:e] & 127).astype(np.float32)
                    val_stream[pos:pos + n] = c["val"][s:e]
                    pos += t * 128
        assert pos == NT * 128
        # wrap idx per call (call = (sb, ch) contiguous span)
        cols = []
        p = 0
        for sb in range(NSUPER):
            blocks = range(sb * SUPER, min((sb + 1) * SUPER, NBLK))
            for ch in range(NCHUNK):
                t = int(T_bc[list(blocks), ch].sum())
                if t == 0:
                    continue
                cols.append(_wrap16(idx_stream[p:p + t * 128]))
                p += t * 128
        c["idx_w"] = np.concatenate(cols, axis=1)
        c["dst_meta"] = dst_stream.reshape(NT, 128).T.copy()
        c["val_meta"] = val_stream.reshape(NT, 128).T.copy()
        for k in ("src", "ldst", "val", "cnt", "starts"):
            del c[k]
    return T_bc, NT, cores


def _prep_decoder(edge_index):
    """Shard decoder edges by edge id; group by (src chunk, dst chunk)."""
    per = N_EDGES // NCORE
    cores = []
    for m in range(NCORE):
        src = edge_index[0, m * per:(m + 1) * per].astype(np.int64)
        dst = edge_index[1, m * per:(m + 1) * per].astype(np.int64)
        g = (src // CHUNK) * NCHUNK + dst // CHUNK
        order = np.argsort(g, kind="stable")
        src, dst, g = src[order], dst[order], g[order]
        eid = m * per + order  # global edge ids in stream order
        cnt = np.bincount(g, minlength=16)
        starts = np.zeros(17, np.int64)
        np.cumsum(cnt, out=starts[1:])
        cores.append(dict(src=src, dst=dst, eid=eid, cnt=cnt, starts=starts))

    cnt_max = np.maximum.reduce([c["cnt"] for c in cores])
    T_g = -(-cnt_max // 128)
    DEC_NT = int(T_g.sum())
    # call list: (group, col_base(tiles), tiles)
    calls = []
    base = 0
    for g in range(16):
        t = int(T_g[g])
        off = 0
        while off < t:
            tt = min(DEC_CALL_TILES, t - off)
            calls.append((g, base + off, tt))
            off += tt
        base += t

    for c in cores:
        src_s = np.zeros(DEC_NT * 128, np.int16)
        dst_s = np.zeros(DEC_NT * 128, np.int16)
        emap = np.full(DEC_NT * 128, -1, np.int64)
        pos = 0
        for g in range(16):
            s, e = c["starts"][g], c["starts"][g + 1]
            n = e - s
            gs, gd = g // NCHUNK, g % NCHUNK
            src_s[pos:pos + n] = (c["src"][s:e] - gs * CHUNK).astype(np.int16)
            dst_s[pos:pos + n] = (c["dst"][s:e] - gd * CHUNK).astype(np.int16)
            emap[pos:pos + n] = c["eid"][s:e]
            pos += int(T_g[g]) * 128
        assert pos == DEC_NT * 128
        # wrap per call
        sc, dc = [], []
        for (g, cb, tt) in calls:
            span = slice(cb * 128, (cb + tt) * 128)
            sc.append(_wrap16(src_s[span]))
            dc.append(_wrap16(dst_s[span]))
        c["src_w"] = np.concatenate(sc, axis=1)
        c["dst_w"] = np.concatenate(dc, axis=1)
        # emap as [128, DEC_NT]: value at [p, t] is edge at pos t*128+p
        c["emap"] = emap.reshape(DEC_NT, 128).T.copy()
        for k in ("src", "dst", "eid", "cnt", "starts"):
            del c[k]
    return T_g, DEC_NT, calls, cores


def _build(T_bc, NT, DEC_NT, dec_calls, idx_cols, dec_cols):
    nc = bacc.Bacc("TRN2", target_bir_lowering=False, debug=False,
                   enable_asserts=True, num_devices=NCORE)

    xT_d = nc.dram_tensor("xT", [D_IN, NODES_PAD], BF16, kind="ExternalInput")
    w1_d = nc.dram_tensor("W1", [D_IN, D_HID], BF16, kind="ExternalInput")
    w2_d = nc.dram_tensor("W2", [D_HID, D_EMB], BF16, kind="ExternalInput")
    b1t_d = nc.dram_tensor("b1t", [128, D_HID], BF16, kind="ExternalInput")
    b2t_d = nc.dram_tensor("b2t", [128, D_EMB], BF16, kind="ExternalInput")
    iota_d = nc.dram_tensor("iota", [128, 128], BF16, kind="ExternalInput")
    ident_d = nc.dram_tensor("ident", [128, 128], BF16, kind="ExternalInput")
    idx_d = nc.dram_tensor("idx", [128, idx_cols], I16, kind="ExternalInput")
    dstm_d = nc.dram_tensor("dstm", [128, NT], F32, kind="ExternalInput")
    valm_d = nc.dram_tensor("valm", [128, NT], F32, kind="ExternalInput")
    dsrc_d = nc.dram_tensor("dsrc", [128, dec_cols], I16, kind="ExternalInput")
    ddst_d = nc.dram_tensor("ddst", [128, dec_cols], I16, kind="ExternalInput")
    scores_d = nc.dram_tensor("scores", [128, DEC_NT], F32, kind="ExternalOutput")

    # internal DRAM
    h1pad_d = nc.dram_tensor("h1pad", [NODES_PAD, 128], BF16, kind="Internal")
    z1_d = nc.dram_tensor("z1", [ZROWS, 128], BF16, kind="Internal")
    h2loc_d = nc.dram_tensor("h2loc", [ZROWS, D_EMB], BF16, kind="Internal")
    h2full_d = nc.dram_tensor("h2full", [N_NODES, D_EMB], BF16,
                              kind="Internal", addr_space="Shared")
    table2_d = nc.dram_tensor("table2", [NODES_PAD, 128], BF16, kind="Internal")
    z2loc_d = nc.dram_tensor("z2loc", [ZROWS, D_EMB], F32, kind="Internal")
    z2full_d = nc.dram_tensor("z2full", [N_NODES, D_EMB], F32,
                              kind="Internal", addr_space="Shared")
    table3_d = nc.dram_tensor("table3", [NODES_PAD, D_HID], F32, kind="Internal")

    rg = [list(range(NCORE))]

    def _expand(tc, pool_, src_d, dst_d, width, dt):
        """Copy [N_NODES, width] rows into the 256B-stride table cols 0:width
        via SBUF bounce tiles."""
        ROWS = 12500  # per bounce chunk: 12500 = 100000/8; not /128 — use 8192
        R = 8192
        for r0 in range(0, N_NODES, R):
            n = min(R, N_NODES - r0)
            nb = -(-n // 128)
            t = pool_.tile([128, nb, width], dt, tag="expand")
            if n % 128:
                # last chunk: 100000 = 12*8192 + 1696; 1696 = 13*128 + 32
                full = n // 128
                if full:
                    nc.sync.dma_start(
                        t[:, 0:full, :],
                        src_d[r0:r0 + full * 128, :]
                        .rearrange("(j p) f -> p j f", p=128))
                rem = n - full * 128
                nc.sync.dma_start(t[0:rem, full, :], src_d[r0 + full * 128:r0 + n, :])
                if full:
                    nc.sync.dma_start(
                        dst_d[r0:r0 + full * 128, 0:width]
                        .rearrange("(j p) f -> p j f", p=128),
                        t[:, 0:full, :])
                nc.sync.dma_start(dst_d[r0 + full * 128:r0 + n, 0:width],
                                  t[0:rem, full, :])
            else:
                nc.sync.dma_start(
                    t[:], src_d[r0:r0 + n, :]
                    .rearrange("(j p) f -> p j f", p=128))
                nc.sync.dma_start(
                    dst_d[r0:r0 + n, 0:width]
                    .rearrange("(j p) f -> p j f", p=128),
                    t[:])

    # per-call tile counts for spmm gathers
    def spmm_calls():
        out = []
        for sb in range(NSUPER):
            blocks = list(range(sb * SUPER, min((sb + 1) * SUPER, NBLK)))
            for ch in range(NCHUNK):
                t = int(T_bc[blocks, ch].sum())
                if t:
                    out.append((sb, ch, blocks, t))
        return out

    CALLS = spmm_calls()
    # global tile base per call, and per (b, ch) local offset within call
    call_tile_base = {}
    tb = 0
    for (sb, ch, blocks, t) in CALLS:
        call_tile_base[(sb, ch)] = tb
        tb += t
    assert tb == NT

    def spmm_phase(tc, pool, table_ap, out_w, bias_tile, relu, out_dtype,
                   z_out_d, tag):
        nc_ = tc.nc
        with ExitStack() as ctx:
            lp = ctx.enter_context(tc.tile_pool(name=f"sp_{tag}", bufs=3))
            gp = ctx.enter_context(tc.tile_pool(name=f"g_{tag}", bufs=8))
            pp = ctx.enter_context(
                tc.tile_pool(name=f"ps_{tag}", bufs=6, space="PSUM"))
            iota_sb = pool.tile([128, 128], BF16, tag="iota")
            ident_sb = pool.tile([128, 128], BF16, tag="ident")
            bt_sb = pool.tile([128, out_w], BF16, tag=f"bt_{tag}")
            nc_.sync.dma_start(iota_sb[:], iota_d[:])
            nc_.sync.dma_start(ident_sb[:], ident_d[:])
            nc_.sync.dma_start(bt_sb[:], bias_tile[:])

            icol = 0
            for sb in range(NSUPER):
                blocks = list(range(sb * SUPER, min((sb + 1) * SUPER, NBLK)))
                sb_tiles = int(T_bc[blocks, :].sum())
                sb_tile0 = call_tile_base[(sb, [ch for ch in range(NCHUNK)
                                                if (sb, ch) in call_tile_base][0])]
                # contiguous metadata slice for this superblock
                dst_sb = lp.tile([128, sb_tiles], F32, tag="dstm")
                val_sb = lp.tile([128, sb_tiles], F32, tag="valm")
                nc_.sync.dma_start(dst_sb[:], dstm_d[:, sb_tile0:sb_tile0 + sb_tiles])
                nc_.sync.dma_start(val_sb[:], valm_d[:, sb_tile0:sb_tile0 + sb_tiles])
                idx_sb = lp.tile([128, sb_tiles * 8], I16, tag="idx")
                nc_.sync.dma_start(idx_sb[:], idx_d[:, icol:icol + sb_tiles * 8])

                gath = {}
                ic_local = 0
                for ch in range(NCHUNK):
                    if (sb, ch) not in call_tile_base:
                        continue
                    t = int(T_bc[blocks, ch].sum())
                    rows = min(CHUNK, NODES_PAD - ch * CHUNK)
                    g = gp.tile([128, t, 128], BF16, tag="gath")
                    nc_.gpsimd.dma_gather(
                        out_ap=g[:],
                        in_ap=table_ap[ch * CHUNK:ch * CHUNK + rows, :],
                        idxs_ap=idx_sb[:, ic_local:ic_local + t * 8],
                        num_idxs=t * 128,
                        num_idxs_reg=t * 128,
                        elem_size=128,
                        single_packet=False,
                    )
                    gath[ch] = g
                    ic_local += t * 8
                icol += sb_tiles * 8

                zwide = lp.tile([128, len(blocks), out_w], out_dtype, tag="zw")
                for bi, b in enumerate(blocks):
                    ps = pp.tile([128, out_w], F32, tag="ps")
                    first = True
                    for ch in range(NCHUNK):
                        if (sb, ch) not in call_tile_base:
                            continue
                        # local tile offset of block b within call (sb, ch)
                        off = int(T_bc[blocks[:bi], ch].sum()) if bi else 0
                        gtile0 = call_tile_base[(sb, ch)] + off
                        for ti in range(int(T_bc[b, ch])):
                            mcol = gtile0 + ti - sb_tile0
                            oh = lp.tile([128, 128], BF16, tag="oh")
                            nc_.vector.tensor_scalar(
                                oh[:], iota_sb[:],
                                dst_sb[:, mcol:mcol + 1],
                                val_sb[:, mcol:mcol + 1],
                                mybir.AluOpType.is_equal,
                                mybir.AluOpType.mult,
                            )
                            nc_.tensor.matmul(
                                ps[:], oh[:], gath[ch][:, off + ti, 0:out_w],
                                start=first, stop=False)
                            first = False
                    nc_.tensor.matmul(ps[:], ident_sb[:], bt_sb[:],
                                      start=first, stop=True)
                    func = (mybir.ActivationFunctionType.Relu if relu
                            else mybir.ActivationFunctionType.Copy)
                    nc_.scalar.activation(zwide[:, bi, :], ps[:], func)
                nc_.sync.dma_start(
                    z_out_d[sb * SUPER * 128:
                            (sb * SUPER + len(blocks)) * 128, :]
                    .rearrange("(j p) f -> p j f", p=128),
                    zwide[:],
                )

    with tile.TileContext(nc) as tc:
        with ExitStack() as octx:
            pool = octx.enter_context(tc.tile_pool(name="const", bufs=1))

            # ---- Phase A: h1 full = x @ W1 (every core, all nodes) ----
            with ExitStack() as ctx:
                ap = ctx.enter_context(tc.tile_pool(name="pA", bufs=3))
                app = ctx.enter_context(
                    tc.tile_pool(name="pAp", bufs=8, space="PSUM"))
                w1_sb = pool.tile([128, 2, D_HID], BF16, tag="w1")
                nc.sync.dma_start(
                    w1_sb[:], w1_d[:].rearrange("(k p) f -> p k f", p=128))
                PB = 8  # node blocks per panel
                for p0 in range(0, 782, PB):
                    nb = min(PB, 782 - p0)
                    n0 = p0 * 128
                    xt0 = ap.tile([128, nb * 128], BF16, tag="xt0")
                    xt1 = ap.tile([128, nb * 128], BF16, tag="xt1")
                    nc.sync.dma_start(xt0[:], xT_d[0:128, n0:n0 + nb * 128])
                    nc.sync.dma_start(xt1[:], xT_d[128:256, n0:n0 + nb * 128])
                    hw = ap.tile([128, nb, D_HID], BF16, tag="hw")
                    for j in range(nb):
                        ps = app.tile([128, D_HID], F32, tag="psA")
                        nc.tensor.matmul(ps[:], xt0[:, j * 128:(j + 1) * 128],
                                         w1_sb[:, 0, :], start=True, stop=False)
                        nc.tensor.matmul(ps[:], xt1[:, j * 128:(j + 1) * 128],
                                         w1_sb[:, 1, :], start=False, stop=True)
                        nc.scalar.activation(hw[:, j, :], ps[:],
                                             mybir.ActivationFunctionType.Copy)
                    nc.sync.dma_start(
                        h1pad_d[n0:n0 + nb * 128, 0:D_HID]
                        .rearrange("(j p) f -> p j f", p=128),
                        hw[:],
                    )

            # ---- calibration phases (env CALIB=1) ----
            if os.environ.get("CALIB"):
                with ExitStack() as ctx:
                    zp = ctx.enter_context(tc.tile_pool(name="pZ", bufs=4))
                    zidx = zp.tile([128, 256], I16)
                    nc.sync.dma_start(zidx[:], idx_d[:, 0:256])
                    iota_sb = zp.tile([128, 128], BF16)
                    nc.sync.dma_start(iota_sb[:], iota_d[:])
                    vm = zp.tile([128, 512], F32)
                    nc.sync.dma_start(vm[:], valm_d[:, 0:512])
                    h1v = h1pad_d[:].rearrange("n (a f) -> (n a) f", a=1)
                    h1w = h1pad_d[:].rearrange("(n a) f -> n (a f)", a=2)
                    for r in range(300):  # Z1: 4096 x 256B
                        g = zp.tile([128, 32, 128], BF16, tag="z1g")
                        nc.gpsimd.dma_gather(out_ap=g[:], in_ap=h1v[0:32768, :], idxs_ap=zidx[:],
                                             num_idxs=4096, num_idxs_reg=4096, elem_size=128,
                                             single_packet=False)
                    for r in range(300):  # Z2: 2048 x 512B
                        g = zp.tile([128, 16, 256], BF16, tag="z2g")
                        nc.gpsimd.dma_gather(out_ap=g[:], in_ap=h1w[0:32768, :], idxs_ap=zidx[:, 0:128],
                                             num_idxs=2048, num_idxs_reg=2048, elem_size=256,
                                             single_packet=False)
                    for r in range(300):  # Z3: 4096 x 512B
                        g = zp.tile([128, 32, 256], BF16, tag="z3g")
                        nc.gpsimd.dma_gather(out_ap=g[:], in_ap=h1w[0:32768, :], idxs_ap=zidx[:],
                                             num_idxs=4096, num_idxs_reg=4096, elem_size=256,
                                             single_packet=False)
                    for r in range(300):  # Z4: 4096 x 256B rotating queues
                        g = zp.tile([128, 32, 128], BF16, tag="z4g")
                        nc.gpsimd.dma_gather(out_ap=g[:], in_ap=h1v[0:32768, :], idxs_ap=zidx[:],
                                             num_idxs=4096, num_idxs_reg=4096, elem_size=128,
                                             single_packet=False)
                    for r in range(1200):  # Z5: 1024 x 256B single_packet
                        g = zp.tile([128, 8, 128], BF16, tag="z5g")
                        nc.gpsimd.dma_gather(out_ap=g[:], in_ap=h1v[0:32768, :], idxs_ap=zidx[:, 0:64],
                                             num_idxs=1024, num_idxs_reg=1024, elem_size=128,
                                             single_packet=True)
                    for r in range(1000):  # Z6: TS [128,32] 2-op
                        oh = zp.tile([128, 32], BF16, tag="z6oh")
                        nc.vector.tensor_scalar(oh[:], iota_sb[:, 0:32], vm[:, r % 512:r % 512 + 1],
                                                vm[:, r % 512:r % 512 + 1],
                                                mybir.AluOpType.is_equal, mybir.AluOpType.mult)
                    for r in range(1000):  # Z7: TS [128,128] 1-op f32->bf16
                        oh = zp.tile([128, 128], BF16, tag="z7oh")
                        nc.vector.tensor_scalar(oh[:], iota_sb[:], vm[:, r % 512:r % 512 + 1],
                                                None, mybir.AluOpType.is_equal)
                    for r in range(1000):  # Z8: TS [128,128] 2-op f32 in (same as spmm)
                        oh = zp.tile([128, 128], BF16, tag="z8oh")
                        nc.vector.tensor_scalar(oh[:], iota_sb[:], vm[:, r % 512:r % 512 + 1],
                                                vm[:, r % 512:r % 512 + 1],
                                                mybir.AluOpType.is_equal, mybir.AluOpType.mult)

            # ---- Phase B: SPMM1 -> z1 (relu) ----
            spmm_phase(tc, pool, h1pad_d[:], D_HID, b1t_d, True, BF16,
                       z1_d[:, 0:D_HID], "b")

            # ---- Phase C: h2 = z1 @ W2; AllGather; expand to table2 ----
            with ExitStack() as ctx:
                cp = ctx.enter_context(tc.tile_pool(name="pC", bufs=3))
                cpp = ctx.enter_context(
                    tc.tile_pool(name="pCp", bufs=4, space="PSUM"))
                z1T = cp.tile([128, ZROWS], BF16, tag="z1T")
                nc.sync.dma_start(z1T[:], z1_d[:], transpose=True)
                w2_sb = pool.tile([D_HID, D_EMB], BF16, tag="w2")
                nc.sync.dma_start(w2_sb[:], w2_d[:])
                PB = 8
                for p0 in range(0, NBLK, PB):
                    nb = min(PB, NBLK - p0)
                    hw = cp.tile([128, nb, D_EMB], BF16, tag="h2w")
                    for j in range(nb):
                        b = p0 + j
                        ps = cpp.tile([128, D_EMB], F32, tag="psC")
                        nc.tensor.matmul(ps[:], z1T[0:D_HID, b * 128:(b + 1) * 128],
                                         w2_sb[:], start=True, stop=True)
                        nc.scalar.activation(hw[:, j, :], ps[:],
                                             mybir.ActivationFunctionType.Copy)
                    nc.sync.dma_start(
                        h2loc_d[p0 * 128:(p0 + nb) * 128, :]
                        .rearrange("(j p) f -> p j f", p=128),
                        hw[:],
                    )
                nc.gpsimd.collective_compute(
                    "AllGather", mybir.AluOpType.bypass, replica_groups=rg,
                    ins=[h2loc_d[0:P_NODES, :]], outs=[h2full_d.ap()])
                _expand(tc, cp, h2full_d, table2_d, D_EMB, BF16)

            # ---- Phase D: SPMM2 -> z2 (no relu, f32 out) ----
            spmm_phase(tc, pool, table2_d[:], D_EMB, b2t_d, False, F32,
                       z2loc_d, "d")

            # AllGather z2, expand to f32 table3
            nc.gpsimd.collective_compute(
                "AllGather", mybir.AluOpType.bypass, replica_groups=rg,
                ins=[z2loc_d[0:P_NODES, :]], outs=[z2full_d.ap()])
            with ExitStack() as ctx:
                xp = ctx.enter_context(tc.tile_pool(name="pX", bufs=4))
                _expand(tc, xp, z2full_d, table3_d, D_EMB, F32)

            # ---- Phase E: decoder ----
            with ExitStack() as ctx:
                ep = ctx.enter_context(tc.tile_pool(name="pE", bufs=4))
                icol = 0
                for (g, cb, tt) in dec_calls:
                    gs, gd = g // NCHUNK, g % NCHUNK
                    si = ep.tile([128, tt * 8], I16, tag="si")
                    di = ep.tile([128, tt * 8], I16, tag="di")
                    nc.sync.dma_start(si[:], dsrc_d[:, icol:icol + tt * 8])
                    nc.sync.dma_start(di[:], ddst_d[:, icol:icol + tt * 8])
                    icol += tt * 8
                    A = ep.tile([128, tt, D_HID], F32, tag="A")
                    B = ep.tile([128, tt, D_HID], F32, tag="B")
                    rows_s = min(CHUNK, NODES_PAD - gs * CHUNK)
                    rows_d = min(CHUNK, NODES_PAD - gd * CHUNK)
                    nc.gpsimd.dma_gather(
                        out_ap=A[:], in_ap=table3_d[gs * CHUNK:gs * CHUNK + rows_s, :],
                        idxs_ap=si[:], num_idxs=tt * 128, num_idxs_reg=tt * 128,
                        elem_size=D_HID, single_packet=False)
                    nc.gpsimd.dma_gather(
                        out_ap=B[:], in_ap=table3_d[gd * CHUNK:gd * CHUNK + rows_d, :],
                        idxs_ap=di[:], num_idxs=tt * 128, num_idxs_reg=tt * 128,
                        elem_size=D_HID, single_packet=False)
                    prod = ep.tile([128, tt, D_EMB], F32, tag="prod")
                    nc.vector.tensor_tensor(
                        prod[:], A[:, :, 0:D_EMB], B[:, :, 0:D_EMB],
                        mybir.AluOpType.mult)
                    red = ep.tile([128, tt], F32, tag="red")
                    nc.vector.tensor_reduce(
                        red[:], prod[:], mybir.AxisListType.X,
                        mybir.AluOpType.add)
                    nc.sync.dma_start(scores_d[:, cb:cb + tt], red[:])

    nc.compile()
    return nc


def kernel(x, adj_src, adj_dst, adj_val, edge_index, W1, b1, W2, b2):
    global LAST_RESULT
    x = np.asarray(x, np.float32)
    adj_src = np.asarray(adj_src, np.int32)
    adj_dst = np.asarray(adj_dst, np.int32)
    adj_val = np.asarray(adj_val, np.float32)
    edge_index = np.asarray(edge_index, np.int32)
    W1 = np.asarray(W1, np.float32)
    b1 = np.asarray(b1, np.float32)
    W2 = np.asarray(W2, np.float32)
    b2 = np.asarray(b2, np.float32)

    T_bc, NT, spmm_cores = _prep_spmm(adj_src, adj_dst, adj_val)
    T_g, DEC_NT, dec_calls, dec_cores = _prep_decoder(edge_index)
    idx_cols = spmm_cores[0]["idx_w"].shape[1]
    dec_cols = dec_cores[0]["src_w"].shape[1]

    nc = _build(T_bc, NT, DEC_NT, dec_calls, idx_cols, dec_cols)

    xT = np.zeros((D_IN, NODES_PAD), BF)
    xT[:, :N_NODES] = x.T.astype(BF)
    iota = np.tile(np.arange(128, dtype=BF)[None, :], (128, 1))
    ident = np.eye(128, dtype=BF)
    b1t = np.tile(b1.astype(BF)[None, :], (128, 1))
    b2t = np.tile(b2.astype(BF)[None, :], (128, 1))
    common = {
        "xT": xT, "W1": W1.astype(BF), "W2": W2.astype(BF),
        "b1t": b1t, "b2t": b2t, "iota": iota, "ident": ident,
    }
    in_maps = []
    for m in range(NCORE):
        in_maps.append({
            **common,
            "idx": spmm_cores[m]["idx_w"],
            "dstm": spmm_cores[m]["dst_meta"],
            "valm": spmm_cores[m]["val_meta"],
            "dsrc": dec_cores[m]["src_w"],
            "ddst": dec_cores[m]["dst_w"],
        })

    res = bass_utils.run_bass_kernel_spmd(
        nc, in_maps, core_ids=list(range(NCORE)),
        trace=bool(os.environ.get("BASS_TRACE")))
    LAST_RESULT = res

    scores = np.zeros(N_EDGES, np.float32)
    for m in range(NCORE):
        out = res.results[m]["scores"]          # [128, DEC_NT]
        emap = dec_cores[m]["emap"]             # [128, DEC_NT]
        valid = emap >= 0
        scores[emap[valid]] = out[valid]
    return scores



# revision 11
# speedup vs baseline: 5.1970x; 5.1970x over previous
"""GCN link predictor on 8 TRN2 NeuronCores.

Strategy (1D node partition, dst-sharded SPMM, pull-mode gathers):
  - x is replicated (transposed, bf16); every core computes the full
    h1 = x @ W1 gather table [100096, 128] bf16 (256B rows).
  - adjacency edges are sharded by dst owner (12500 nodes/core); within a
    core, edges are grouped by (128-row dst block, 32768-row src chunk) so
    each dma_gather call uses int16 indices against a single chunk base.
  - segment-sum = PE matmul with one-hots built in BATCHED DVE
    tensor_tensor ops (broadcast APs) per superblock — avoids the ~2us
    fixed cost of per-tile tensor_scalar.
  - layer 2: h2 = z1 @ W2 locally, AllGather h2 (bf16), expand into a
    256B-row table, same SPMM -> z2 (bf16).
  - decoder: edges sharded by dst owner, sorted by (dst window, src
    chunk).  dst side = PE select from SBUF-resident local z2 windows
    (one-hot streamed from host, zero descriptors); src side = transposed
    dma_gather (A^T columns); DVE product + PE ones-reduction into PSUM
    score rows; periodic flush to SBUF.
"""
import sys
import os

sys.path.insert(0, "/opt/trn_rl_repo")

import numpy as np
import ml_dtypes
from contextlib import ExitStack

from concourse import bass, bacc, tile, bass_utils
import concourse.mybir as mybir


def _install_ntff_hook():
    """Provide antenv.axon_hooks (missing in this image) so that
    run_bass_kernel_spmd(trace=True) can capture NTFF profiles via the
    axon PJRT .so — mirrors trn_agent_boot's ctypes shim."""
    if "antenv.axon_hooks" in sys.modules:
        return
    import types, ctypes, contextlib
    import antenv

    mod = types.ModuleType("antenv.axon_hooks")
    holder = {}
    mod.set_axon_ntff_profile_hook = lambda h: holder.__setitem__("h", h)
    mod.get_axon_ntff_profile_hook = lambda: holder.get("h")
    sys.modules["antenv.axon_hooks"] = mod
    antenv.axon_hooks = mod

    so_path = "/opt/axon/libaxon_pjrt.so"
    if not os.path.exists(so_path):
        return
    lib = ctypes.CDLL(so_path)
    if not hasattr(lib, "axon_start_nrt_profile"):
        return
    lib.axon_start_nrt_profile.argtypes = [ctypes.POINTER(ctypes.c_int64),
                                           ctypes.c_size_t]
    lib.axon_start_nrt_profile.restype = ctypes.c_int64
    lib.axon_stop_nrt_profile.argtypes = [ctypes.c_char_p]
    lib.axon_stop_nrt_profile.restype = ctypes.c_int64

    @contextlib.contextmanager
    def _hook(output_dir, device_ids):
        import jax
        jax.devices()
        if device_ids:
            ids = (ctypes.c_int64 * len(device_ids))(*device_ids)
            rc = lib.axon_start_nrt_profile(ids, len(device_ids))
        else:
            rc = lib.axon_start_nrt_profile(None, 0)
        if rc != 0:
            raise RuntimeError(f"axon_start_nrt_profile rc={rc}")
        try:
            yield
        finally:
            n = lib.axon_stop_nrt_profile(str(output_dir).encode())
            print(f"profile: {n} file(s) written to {output_dir}",
                  file=sys.stderr)

    mod.set_axon_ntff_profile_hook(_hook)


_install_ntff_hook()

F32 = mybir.dt.float32
BF16 = mybir.dt.bfloat16
I16 = mybir.dt.int16
BF = ml_dtypes.bfloat16

N_NODES = 100000
D_IN = 256
D_HID = 64
D_EMB = 32
ADJ_NNZ = 3200000
N_EDGES = 2000000
NCORE = 8
P_NODES = N_NODES // NCORE          # 12500
NODES_PAD = 782 * 128               # 100096
ZROWS = 98 * 128                    # 12544 (per-core z rows, padded)
NBLK = 98                           # dst blocks per core (last has 84 rows)
CHUNK = 32768
NCHUNK = 4
SUPER = 2                           # dst blocks per superblock
NSUPER = (NBLK + SUPER - 1) // SUPER
DEC_SUPERW = 13                     # decoder dst windows per gather group
DEC_NSW = (NBLK + DEC_SUPERW - 1) // DEC_SUPERW

LAST_RESULT = None                  # BassKernelResults of the last run


def _wrap16(idx):
    """idx j -> partition j%16, col j//16, replicated to 128 partitions."""
    n = len(idx)
    assert n % 16 == 0
    a = idx.reshape(n // 16, 16).T
    return np.tile(a, (8, 1)).astype(np.int16)


def _prep_spmm(adj_src, adj_dst, adj_val):
    """Shard + sort adjacency; build per-core gather/meta streams.

    Returns (T_bc [98,4] common tile counts, NT, per-core dict with idx_w,
    dst_meta (bf16), val_meta (bf16)).
    """
    owner = adj_dst // P_NODES
    cores = []
    for m in range(NCORE):
        sel = owner == m
        src = adj_src[sel].astype(np.int64)
        ldst = (adj_dst[sel] - m * P_NODES).astype(np.int64)
        val = adj_val[sel]
        blk = ldst >> 7
        chk = src // CHUNK
        order = np.lexsort((chk, blk))
        src, ldst, val, blk, chk = (a[order] for a in (src, ldst, val, blk, chk))
        key = blk * NCHUNK + chk
        cnt = np.bincount(key, minlength=NBLK * NCHUNK).reshape(NBLK, NCHUNK)
        starts = np.zeros(NBLK * NCHUNK + 1, np.int64)
        np.cumsum(cnt.ravel(), out=starts[1:])
        cores.append(dict(src=src, ldst=ldst, val=val, cnt=cnt, starts=starts))

    cnt_max = np.maximum.reduce([c["cnt"] for c in cores])
    T_bc = -(-cnt_max // 128)  # ceil
    NT = int(T_bc.sum())

    for c in cores:
        idx_stream = np.zeros(NT * 128, np.int16)
        dst_stream = np.zeros(NT * 128, np.float32)
        val_stream = np.zeros(NT * 128, np.float32)
        pos = 0
        for sb in range(NSUPER):
            blocks = range(sb * SUPER, min((sb + 1) * SUPER, NBLK))
            for ch in range(NCHUNK):
                for b in blocks:
                    t = int(T_bc[b, ch])
                    if t == 0:
                        continue
                    s = c["starts"][b * NCHUNK + ch]
                    e = c["starts"][b * NCHUNK + ch + 1]
                    n = e - s
                    idx_stream[pos:pos + n] = (c["src"][s:e] - ch * CHUNK).astype(np.int16)
                    dst_stream[pos:pos + n] = (c["ldst"][s:e] & 127).astype(np.float32)
                    val_stream[pos:pos + n] = c["val"][s:e]
                    pos += t * 128
        assert pos == NT * 128
        # wrap idx per call (call = (sb, ch) contiguous span)
        cols = []
        p = 0
        for sb in range(NSUPER):
            blocks = range(sb * SUPER, min((sb + 1) * SUPER, NBLK))
            for ch in range(NCHUNK):
                t = int(T_bc[list(blocks), ch].sum())
                if t == 0:
                    continue
                cols.append(_wrap16(idx_stream[p:p + t * 128]))
                p += t * 128
        c["idx_w"] = np.concatenate(cols, axis=1)
        c["dst_meta"] = dst_stream.reshape(NT, 128).T.astype(BF).copy()
        c["val_meta"] = val_stream.reshape(NT, 128).T.astype(BF).copy()
        for k in ("src", "ldst", "val", "cnt", "starts"):
            del c[k]
    return T_bc, NT, cores


def _prep_decoder(edge_index):
    """Shard decoder edges by dst owner; group by (dst window, src chunk).

    Per core streams: src idx (wrapped per (superw, chunk) call), one-hot
    of (ldst & 127) as [128, NT*128] bf16, emap for output reordering.
    """
    src_g = edge_index[0].astype(np.int64)
    dst_g = edge_index[1].astype(np.int64)
    owner = dst_g // P_NODES
    cores = []
    for m in range(NCORE):
        sel = owner == m
        src = src_g[sel]
        ldst = dst_g[sel] - m * P_NODES
        eid = np.nonzero(sel)[0]
        w = ldst >> 7                      # 98 windows
        c = src // CHUNK                   # 4 chunks
        sw = w // DEC_SUPERW
        order = np.lexsort((w, c, sw))
        src, ldst, eid, w, c, sw = (a[order] for a in (src, ldst, eid, w, c, sw))
        key = (w * NCHUNK + c)
        cnt = np.bincount(key, minlength=NBLK * NCHUNK).reshape(NBLK, NCHUNK)
        starts = np.zeros(NBLK * NCHUNK + 1, np.int64)
        # starts in (sw, c, w) stream order
        sizes = []
        for isw in range(DEC_NSW):
            ws = range(isw * DEC_SUPERW, min((isw + 1) * DEC_SUPERW, NBLK))
            for ch in range(NCHUNK):
                for iw in ws:
                    sizes.append(cnt[iw, ch])
        assert sum(sizes) == len(src)
        cores.append(dict(src=src, ldst=ldst, eid=eid, cnt=cnt))

    cnt_max = np.maximum.reduce([c["cnt"] for c in cores])
    T_wc = -(-cnt_max // 128)              # [98, 4] tiles
    DEC_NT = int(T_wc.sum())

    for cd in cores:
        idx_stream = np.zeros(DEC_NT * 128, np.int16)
        oh = np.zeros((128, DEC_NT * 128), BF)
        emap = np.full(DEC_NT * 128, -1, np.int64)
        # per-(w,c) start offsets in this core's sorted arrays
        starts = {}
        pos0 = 0
        for isw in range(DEC_NSW):
            ws = range(isw * DEC_SUPERW, min((isw + 1) * DEC_SUPERW, NBLK))
            for ch in range(NCHUNK):
                for iw in ws:
                    starts[(iw, ch)] = pos0
                    pos0 += int(cd["cnt"][iw, ch])
        pos = 0
        for isw in range(DEC_NSW):
            ws = range(isw * DEC_SUPERW, min((isw + 1) * DEC_SUPERW, NBLK))
            for ch in range(NCHUNK):
                for iw in ws:
                    t = int(T_wc[iw, ch])
                    if t == 0:
                        continue
                    s = starts[(iw, ch)]
                    n = int(cd["cnt"][iw, ch])
                    idx_stream[pos:pos + n] = (cd["src"][s:s + n] - ch * CHUNK).astype(np.int16)
                    loff = (cd["ldst"][s:s + n] & 127).astype(np.int64)
                    oh[loff, np.arange(pos, pos + n)] = BF(1.0)
                    emap[pos:pos + n] = cd["eid"][s:s + n]
                    pos += t * 128
        assert pos == DEC_NT * 128
        # wrap idx per (sw, ch) call
        cols = []
        p = 0
        for isw in range(DEC_NSW):
            ws = list(range(isw * DEC_SUPERW, min((isw + 1) * DEC_SUPERW, NBLK)))
            for ch in range(NCHUNK):
                t = int(T_wc[ws, ch].sum())
                if t == 0:
                    continue
                cols.append(_wrap16(idx_stream[p:p + t * 128]))
                p += t * 128
        cd["idx_w"] = np.concatenate(cols, axis=1)
        cd["ohdst"] = oh
        cd["emap"] = emap
        for k in ("src", "ldst", "eid", "cnt"):
            del cd[k]
    return T_wc, DEC_NT, cores


def _build(T_bc, NT, T_wc, DEC_NT, idx_cols, dec_idx_cols):
    nc = bacc.Bacc("TRN2", target_bir_lowering=False, debug=False,
                   num_devices=NCORE)

    xT_d = nc.dram_tensor("xT", [D_IN, NODES_PAD], BF16, kind="ExternalInput")
    w1_d = nc.dram_tensor("W1", [D_IN, D_HID], BF16, kind="ExternalInput")
    w2_d = nc.dram_tensor("W2", [D_HID, D_EMB], BF16, kind="ExternalInput")
    b1t_d = nc.dram_tensor("b1t", [128, D_HID], BF16, kind="ExternalInput")
    b2t_d = nc.dram_tensor("b2t", [128, D_EMB], BF16, kind="ExternalInput")
    iota_d = nc.dram_tensor("iota", [128, 128], BF16, kind="ExternalInput")
    ident_d = nc.dram_tensor("ident", [128, 128], BF16, kind="ExternalInput")
    idx_d = nc.dram_tensor("idx", [128, idx_cols], I16, kind="ExternalInput")
    dstm_d = nc.dram_tensor("dstm", [128, NT], BF16, kind="ExternalInput")
    valm_d = nc.dram_tensor("valm", [128, NT], BF16, kind="ExternalInput")
    didx_d = nc.dram_tensor("didx", [128, dec_idx_cols], I16, kind="ExternalInput")
    doh_d = nc.dram_tensor("doh", [128, DEC_NT * 128], BF16, kind="ExternalInput")
    scores_d = nc.dram_tensor("scores", [128, DEC_NT], F32,
                              kind="ExternalOutput")

    # internal DRAM
    h1pad_d = nc.dram_tensor("h1pad", [NODES_PAD, 128], BF16, kind="Internal")
    z1_d = nc.dram_tensor("z1", [ZROWS, 128], BF16, kind="Internal")
    h2loc_d = nc.dram_tensor("h2loc", [ZROWS, D_EMB], BF16, kind="Internal")
    h2full_d = nc.dram_tensor("h2full", [N_NODES, D_EMB], BF16,
                              kind="Internal", addr_space="Shared")
    table2_d = nc.dram_tensor("table2", [NODES_PAD, 128], BF16, kind="Internal")
    z2loc_d = nc.dram_tensor("z2loc", [ZROWS, D_EMB], BF16, kind="Internal")
    z2full_d = nc.dram_tensor("z2full", [N_NODES, D_EMB], BF16,
                              kind="Internal", addr_space="Shared")
    table3_d = nc.dram_tensor("table3", [NODES_PAD, 128], BF16, kind="Internal")

    rg = [list(range(NCORE))]

    def _expand(tc, pool_, src_d, dst_d, width, dt):
        """Copy [N_NODES, width] rows into the 256B-stride table cols 0:width
        via SBUF bounce tiles."""
        R = 8192
        for r0 in range(0, N_NODES, R):
            n = min(R, N_NODES - r0)
            nb = -(-n // 128)
            t = pool_.tile([128, nb, width], dt, tag="expand")
            if n % 128:
                full = n // 128
                if full:
                    nc.sync.dma_start(
                        t[:, 0:full, :],
                        src_d[r0:r0 + full * 128, :]
                        .rearrange("(j p) f -> p j f", p=128))
                rem = n - full * 128
                nc.sync.dma_start(t[0:rem, full, :], src_d[r0 + full * 128:r0 + n, :])
                if full:
                    nc.sync.dma_start(
                        dst_d[r0:r0 + full * 128, 0:width]
                        .rearrange("(j p) f -> p j f", p=128),
                        t[:, 0:full, :])
                nc.sync.dma_start(dst_d[r0 + full * 128:r0 + n, 0:width],
                                  t[0:rem, full, :])
            else:
                nc.sync.dma_start(
                    t[:], src_d[r0:r0 + n, :]
                    .rearrange("(j p) f -> p j f", p=128))
                nc.sync.dma_start(
                    dst_d[r0:r0 + n, 0:width]
                    .rearrange("(j p) f -> p j f", p=128),
                    t[:])

    # per-call tile counts for spmm gathers
    def spmm_calls():
        out = []
        for sb in range(NSUPER):
            blocks = list(range(sb * SUPER, min((sb + 1) * SUPER, NBLK)))
            for ch in range(NCHUNK):
                t = int(T_bc[blocks, ch].sum())
                if t:
                    out.append((sb, ch, blocks, t))
        return out

    CALLS = spmm_calls()
    call_tile_base = {}
    tb = 0
    for (sb, ch, blocks, t) in CALLS:
        call_tile_base[(sb, ch)] = tb
        tb += t
    assert tb == NT

    def spmm_phase(tc, pool, table_ap, out_w, bias_tile, relu, out_dtype,
                   z_out_d, tag):
        nc_ = tc.nc
        with ExitStack() as ctx:
            lp = ctx.enter_context(tc.tile_pool(name=f"sp_{tag}", bufs=3))
            ohp = ctx.enter_context(tc.tile_pool(name=f"oh_{tag}", bufs=2))
            gp = ctx.enter_context(tc.tile_pool(name=f"g_{tag}", bufs=8))
            pp = ctx.enter_context(
                tc.tile_pool(name=f"ps_{tag}", bufs=6, space="PSUM"))
            iota_sb = pool.tile([128, 128], BF16, tag="iota")
            ident_sb = pool.tile([128, 128], BF16, tag="ident")
            bt_sb = pool.tile([128, out_w], BF16, tag=f"bt_{tag}")
            nc_.sync.dma_start(iota_sb[:], iota_d[:])
            nc_.sync.dma_start(ident_sb[:], ident_d[:])
            nc_.sync.dma_start(bt_sb[:], bias_tile[:])

            icol = 0
            for sb in range(NSUPER):
                blocks = list(range(sb * SUPER, min((sb + 1) * SUPER, NBLK)))
                sb_tiles = int(T_bc[blocks, :].sum())
                sb_tile0 = call_tile_base[(sb, [ch for ch in range(NCHUNK)
                                                if (sb, ch) in call_tile_base][0])]
                dst_sb = lp.tile([128, sb_tiles], BF16, tag="dstm")
                val_sb = lp.tile([128, sb_tiles], BF16, tag="valm")
                nc_.sync.dma_start(dst_sb[:], dstm_d[:, sb_tile0:sb_tile0 + sb_tiles])
                nc_.sync.dma_start(val_sb[:], valm_d[:, sb_tile0:sb_tile0 + sb_tiles])
                idx_sb = lp.tile([128, sb_tiles * 8], I16, tag="idx")
                nc_.sync.dma_start(idx_sb[:], idx_d[:, icol:icol + sb_tiles * 8])

                gath = {}
                ic_local = 0
                for ch in range(NCHUNK):
                    if (sb, ch) not in call_tile_base:
                        continue
                    t = int(T_bc[blocks, ch].sum())
                    rows = min(CHUNK, NODES_PAD - ch * CHUNK)
                    g = gp.tile([128, t, 128], BF16, tag="gath")
                    nc_.gpsimd.dma_gather(
                        out_ap=g[:],
                        in_ap=table_ap[ch * CHUNK:ch * CHUNK + rows, :],
                        idxs_ap=idx_sb[:, ic_local:ic_local + t * 8],
                        num_idxs=t * 128,
                        num_idxs_reg=t * 128,
                        elem_size=128,
                        single_packet=False,
                    )
                    gath[ch] = g
                    ic_local += t * 8
                icol += sb_tiles * 8

                # batched one-hot build for the whole superblock
                oh_sb = ohp.tile([128, sb_tiles, 128], BF16, tag="oh")
                i_b = iota_sb[:].unsqueeze(1).broadcast_to([128, sb_tiles, 128])
                d_b = dst_sb[:].unsqueeze(2).broadcast_to([128, sb_tiles, 128])
                v_b = val_sb[:].unsqueeze(2).broadcast_to([128, sb_tiles, 128])
                nc_.vector.tensor_tensor(oh_sb[:], i_b, d_b,
                                         mybir.AluOpType.is_equal)
                nc_.vector.tensor_tensor(oh_sb[:], oh_sb[:], v_b,
                                         mybir.AluOpType.mult)

                zwide = lp.tile([128, len(blocks), out_w], out_dtype, tag="zw")
                for bi, b in enumerate(blocks):
                    ps = pp.tile([128, out_w], F32, tag="ps")
                    first = True
                    for ch in range(NCHUNK):
                        if (sb, ch) not in call_tile_base:
                            continue
                        off = int(T_bc[blocks[:bi], ch].sum()) if bi else 0
                        gtile0 = call_tile_base[(sb, ch)] + off
                        for ti in range(int(T_bc[b, ch])):
                            mcol = gtile0 + ti - sb_tile0
                            nc_.tensor.matmul(
                                ps[:], oh_sb[:, mcol, :],
                                gath[ch][:, off + ti, 0:out_w],
                                start=first, stop=False)
                            first = False
                    nc_.tensor.matmul(ps[:], ident_sb[:], bt_sb[:],
                                      start=first, stop=True)
                    func = (mybir.ActivationFunctionType.Relu if relu
                            else mybir.ActivationFunctionType.Copy)
                    nc_.scalar.activation(zwide[:, bi, :], ps[:], func)
                nc_.sync.dma_start(
                    z_out_d[sb * SUPER * 128:
                            (sb * SUPER + len(blocks)) * 128, :]
                    .rearrange("(j p) f -> p j f", p=128),
                    zwide[:],
                )

    with tile.TileContext(nc) as tc:
        with ExitStack() as octx:
            pool = octx.enter_context(tc.tile_pool(name="const", bufs=1))

            # ---- Phase A: h1 full = x @ W1 (every core, all nodes) ----
            with ExitStack() as ctx:
                ap = ctx.enter_context(tc.tile_pool(name="pA", bufs=3))
                app = ctx.enter_context(
                    tc.tile_pool(name="pAp", bufs=8, space="PSUM"))
                w1_sb = pool.tile([128, 2, D_HID], BF16, tag="w1")
                nc.sync.dma_start(
                    w1_sb[:], w1_d[:].rearrange("(k p) f -> p k f", p=128))
                PB = 8  # node blocks per panel
                for p0 in range(0, 782, PB):
                    nb = min(PB, 782 - p0)
                    n0 = p0 * 128
                    xt0 = ap.tile([128, nb * 128], BF16, tag="xt0")
                    xt1 = ap.tile([128, nb * 128], BF16, tag="xt1")
                    nc.sync.dma_start(xt0[:], xT_d[0:128, n0:n0 + nb * 128])
                    nc.sync.dma_start(xt1[:], xT_d[128:256, n0:n0 + nb * 128])
                    hw = ap.tile([128, nb, D_HID], BF16, tag="hw")
                    for j in range(nb):
                        ps = app.tile([128, D_HID], F32, tag="psA")
                        nc.tensor.matmul(ps[:], xt0[:, j * 128:(j + 1) * 128],
                                         w1_sb[:, 0, :], start=True, stop=False)
                        nc.tensor.matmul(ps[:], xt1[:, j * 128:(j + 1) * 128],
                                         w1_sb[:, 1, :], start=False, stop=True)
                        nc.scalar.activation(hw[:, j, :], ps[:],
                                             mybir.ActivationFunctionType.Copy)
                    nc.sync.dma_start(
                        h1pad_d[n0:n0 + nb * 128, 0:D_HID]
                        .rearrange("(j p) f -> p j f", p=128),
                        hw[:],
                    )

            # ---- Phase B: SPMM1 -> z1 (relu) ----
            spmm_phase(tc, pool, h1pad_d[:], D_HID, b1t_d, True, BF16,
                       z1_d[:, 0:D_HID], "b")

            # ---- Phase C: h2 = z1 @ W2; AllGather; expand to table2 ----
            with ExitStack() as ctx:
                cp = ctx.enter_context(tc.tile_pool(name="pC", bufs=3))
                cpp = ctx.enter_context(
                    tc.tile_pool(name="pCp", bufs=4, space="PSUM"))
                z1T = cp.tile([128, ZROWS], BF16, tag="z1T")
                nc.sync.dma_start(z1T[:], z1_d[:], transpose=True)
                w2_sb = pool.tile([D_HID, D_EMB], BF16, tag="w2")
                nc.sync.dma_start(w2_sb[:], w2_d[:])
                PB = 8
                for p0 in range(0, NBLK, PB):
                    nb = min(PB, NBLK - p0)
                    hw = cp.tile([128, nb, D_EMB], BF16, tag="h2w")
                    for j in range(nb):
                        b = p0 + j
                        ps = cpp.tile([128, D_EMB], F32, tag="psC")
                        nc.tensor.matmul(ps[:], z1T[0:D_HID, b * 128:(b + 1) * 128],
                                         w2_sb[:], start=True, stop=True)
                        nc.scalar.activation(hw[:, j, :], ps[:],
                                             mybir.ActivationFunctionType.Copy)
                    nc.sync.dma_start(
                        h2loc_d[p0 * 128:(p0 + nb) * 128, :]
                        .rearrange("(j p) f -> p j f", p=128),
                        hw[:],
                    )
                nc.gpsimd.collective_compute(
                    "AllGather", mybir.AluOpType.bypass, replica_groups=rg,
                    ins=[h2loc_d[0:P_NODES, :]], outs=[h2full_d.ap()])
                _expand(tc, cp, h2full_d, table2_d, D_EMB, BF16)

            # ---- Phase D: SPMM2 -> z2 (no relu, bf16 out) ----
            spmm_phase(tc, pool, table2_d[:], D_EMB, b2t_d, False, BF16,
                       z2loc_d, "d")

            # AllGather z2 (bf16), expand to table3
            nc.gpsimd.collective_compute(
                "AllGather", mybir.AluOpType.bypass, replica_groups=rg,
                ins=[z2loc_d[0:P_NODES, :]], outs=[z2full_d.ap()])
            with ExitStack() as ctx:
                xp = ctx.enter_context(tc.tile_pool(name="pX", bufs=4))
                _expand(tc, xp, z2full_d, table3_d, D_EMB, BF16)

            # ---- Phase E: decoder ----
            GRP = 8
            with ExitStack() as ctx:
                ep = ctx.enter_context(tc.tile_pool(name="pE", bufs=3))
                egp = ctx.enter_context(tc.tile_pool(name="pEg", bufs=4))
                epp = ctx.enter_context(
                    tc.tile_pool(name="pEp", bufs=4, space="PSUM"))
                # local z2 windows, partition = ldst & 127
                z2w_sb = pool.tile([128, NBLK, D_EMB], BF16, tag="z2w")
                nc.sync.dma_start(
                    z2w_sb[:],
                    z2loc_d[:].rearrange("(j p) f -> p j f", p=128))
                # scores accumulate in SBUF [128, DEC_NT] (edge = t*128 + p)
                sc_sb = pool.tile([128, DEC_NT], F32, tag="scsb")

                tile_pos = 0          # global tile counter (stream order)
                icol = 0
                for isw in range(DEC_NSW):
                    ws = list(range(isw * DEC_SUPERW,
                                    min((isw + 1) * DEC_SUPERW, NBLK)))
                    for ch in range(NCHUNK):
                        t = int(T_wc[ws, ch].sum())
                        if t == 0:
                            continue
                        didx_sb = ep.tile([128, t * 8], I16, tag="didx")
                        nc.sync.dma_start(didx_sb[:],
                                          didx_d[:, icol:icol + t * 8])
                        icol += t * 8
                        rows = min(CHUNK, NODES_PAD - ch * CHUNK)
                        at = egp.tile([128, t, 128], BF16, tag="at")
                        nc.gpsimd.dma_gather(
                            out_ap=at[:],
                            in_ap=table3_d[ch * CHUNK:ch * CHUNK + rows, :],
                            idxs_ap=didx_sb[:],
                            num_idxs=t * 128, num_idxs_reg=t * 128,
                            elem_size=128, single_packet=False)
                        oh_sb = ep.tile([128, t * 128], BF16, tag="doh")
                        nc.sync.dma_start(
                            oh_sb[:],
                            doh_d[:, tile_pos * 128:(tile_pos + t) * 128])
                        # window of each tile in this call, in stream order
                        wlist = []
                        for iw in ws:
                            wlist += [iw] * int(T_wc[iw, ch])
                        for g0 in range(0, t, GRP):
                            gn = min(GRP, t - g0)
                            bt2 = epp.tile([128, GRP, D_EMB], F32, tag="bt2")
                            for j in range(gn):
                                tl = g0 + j
                                nc.tensor.matmul(
                                    bt2[:, j, :],
                                    oh_sb[:, tl * 128:(tl + 1) * 128],
                                    z2w_sb[:, wlist[tl], :],
                                    start=True, stop=True)
                            prod = ep.tile([128, GRP, D_EMB], BF16, tag="prod")
                            nc.vector.tensor_tensor(
                                prod[:, 0:gn, :], at[:, g0:g0 + gn, 0:D_EMB],
                                bt2[:, 0:gn, :], mybir.AluOpType.mult)
                            nc.vector.tensor_reduce(
                                sc_sb[:, tile_pos + g0:tile_pos + g0 + gn],
                                prod[:, 0:gn, :], mybir.AxisListType.X,
                                mybir.AluOpType.add)
                        tile_pos += t
                nc.sync.dma_start(scores_d[:], sc_sb[:])

    nc.compile()
    return nc


def kernel(x, adj_src, adj_dst, adj_val, edge_index, W1, b1, W2, b2):
    global LAST_RESULT
    x = np.asarray(x, np.float32)
    adj_src = np.asarray(adj_src, np.int32)
    adj_dst = np.asarray(adj_dst, np.int32)
    adj_val = np.asarray(adj_val, np.float32)
    edge_index = np.asarray(edge_index, np.int32)
    W1 = np.asarray(W1, np.float32)
    b1 = np.asarray(b1, np.float32)
    W2 = np.asarray(W2, np.float32)
    b2 = np.asarray(b2, np.float32)

    T_bc, NT, spmm_cores = _prep_spmm(adj_src, adj_dst, adj_val)
    T_wc, DEC_NT, dec_cores = _prep_decoder(edge_index)
    idx_cols = spmm_cores[0]["idx_w"].shape[1]
    dec_idx_cols = dec_cores[0]["idx_w"].shape[1]
    nc = _build(T_bc, NT, T_wc, DEC_NT, idx_cols, dec_idx_cols)

    xT = np.zeros((D_IN, NODES_PAD), BF)
    xT[:, :N_NODES] = x.T.astype(BF)
    iota = np.tile(np.arange(128, dtype=BF)[None, :], (128, 1))
    ident = np.eye(128, dtype=BF)
    b1t = np.tile(b1.astype(BF)[None, :], (128, 1))
    b2t = np.tile(b2.astype(BF)[None, :], (128, 1))
    common = {
        "xT": xT, "W1": W1.astype(BF), "W2": W2.astype(BF),
        "b1t": b1t, "b2t": b2t, "iota": iota, "ident": ident,
    }
    in_maps = []
    for m in range(NCORE):
        in_maps.append({
            **common,
            "idx": spmm_cores[m]["idx_w"],
            "dstm": spmm_cores[m]["dst_meta"],
            "valm": spmm_cores[m]["val_meta"],
            "didx": dec_cores[m]["idx_w"],
            "doh": dec_cores[m]["ohdst"],
        })

    res = bass_utils.run_bass_kernel_spmd(
        nc, in_maps, core_ids=list(range(NCORE)),
        trace=bool(os.environ.get("BASS_TRACE")))
    LAST_RESULT = res

    scores = np.zeros(N_EDGES, np.float32)
    for m in range(NCORE):
        out = res.results[m]["scores"]          # [128, DEC_NT]
        emap = dec_cores[m]["emap"].reshape(DEC_NT, 128).T  # [128, DEC_NT]
        valid = emap >= 0
        scores[emap[valid]] = out[valid]
    return scores
